# revision 1
# baseline (speedup 1.0000x reference)
"""Trainium2 Bass kernel: accepted-token cache gather.

reference(accept_index, out_cache_loc):
    mask = accept_index >= 0; dst = exclusive-prefix-count(mask);
    out[dst[mask]] = out_cache_loc[accept_index[mask]]

With the given input distribution (randint(0, N_CACHE)) every index is
non-negative, so mask is all-true and the op reduces to a pure gather:
out = out_cache_loc[accept_index].

Device strategy (8 NeuronCores):
  - indices sharded 1M per core, table (128MiB) replicated
  - per-element gather via GPSIMD indirect DMA (DmaIndirect1d ucode):
    each instruction gathers 4096 elements (ucode max). The ucode consumes
    indices in snake order (partition-fastest) from a [128, 32] int32 SBUF
    tile, and the dest AP [1, 4096, 1] makes walrus encode 4096 rows x 1
    element into one partition's free dim.
  - host pre-shuffles indices into snake layout; gather outputs then land
    in natural flat order.
"""

import numpy as np

N_ACCEPT = 8388608
N_CACHE = 33554432
N_CORES = 8
SHARD = N_ACCEPT // N_CORES  # 1048576
P = 128

GATHER_N = 4096          # indices per indirect DMA (ucode limit)
IDX_W = GATHER_N // P    # 32 per partition
GRP = 32                 # instructions per load/store group
N_INSTR = SHARD // GATHER_N  # 256
N_GRP = N_INSTR // GRP       # 8

_cached = {}

# test-harness knobs (not used by the grading path)
TRACE = False
LAST_RESULTS = None


def _build_bass(
    n_grp: int,
    grp: int,
    n_cache: int,
    reps: int = 1,
    bufs: int = 3,
    gather_n: int = GATHER_N,
    preload_idx: bool = False,
):
    """SPMD program: per-core gather of n_grp*grp*gather_n indices from a
    [n_cache, 1] f32 table.  reps>1 repeats the gather phase (timing only).
    """
    from concourse import bacc, bass, mybir, tile

    nc = bacc.Bacc(
        "TRN2",
        target_bir_lowering=False,
        debug=False,
        enable_asserts=False,
        num_devices=N_CORES,
    )

    idx_w = gather_n // P
    gw = grp * idx_w  # idx free width per group
    idx_d = nc.dram_tensor(
        "idx", [n_grp, P, gw], mybir.dt.int32, kind="ExternalInput"
    )
    tab_d = nc.dram_tensor(
        "table", [n_cache, 1], mybir.dt.float32, kind="ExternalInput"
    )
    out_d = nc.dram_tensor(
        "out", [n_grp, grp, gather_n], mybir.dt.float32, kind="ExternalOutput"
    )

    with tile.TileContext(nc) as tc:
        with (
            tc.tile_pool(name="sbuf", bufs=bufs) as pool,
            tc.tile_pool(name="persist", bufs=1) as ppool,
        ):
            idx_all = (
                ppool.tile([P, n_grp, gw], mybir.dt.int32, name="idx_all", tag="idx_all")
                if preload_idx
                else None
            )

            def phase():
                if preload_idx:
                    nc.sync.dma_start(
                        out=idx_all[:], in_=idx_d.ap().transpose([1, 0, 2])
                    )
                for g0 in range(n_grp):
                    if preload_idx:
                        idx_sb = idx_all[:, g0]
                    else:
                        idx_sb = pool.tile([P, gw], mybir.dt.int32, tag="idx")
                        nc.sync.dma_start(out=idx_sb[:], in_=idx_d.ap()[g0])
                    out_sb = pool.tile([grp, gather_n], mybir.dt.float32, tag="out")
                    for g in range(grp):
                        nc.gpsimd.indirect_dma_start(
                            out=out_sb[g : g + 1, :, None],
                            out_offset=None,
                            in_=tab_d.ap()[:],
                            in_offset=bass.IndirectOffsetOnAxis(
                                ap=idx_sb[:, g * idx_w : (g + 1) * idx_w], axis=0
                            ),
                        )
                    nc.sync.dma_start(out=out_d.ap()[g0], in_=out_sb[:])

            if reps == 1:
                phase()
            else:
                with tc.For_i(0, reps, 1):
                    phase()
    nc.compile()
    return nc


def get_nc(reps: int = 1, bufs: int = 3):
    key = (N_GRP, GRP, N_CACHE, reps, bufs)
    if key not in _cached:
        _cached[key] = _build_bass(*key)
    return _cached[key]


def snake_idx(
    flat_idx32: np.ndarray,
    n_grp: int = N_GRP,
    grp: int = GRP,
    idx_w: int = IDX_W,
):
    """[n] int32 -> [n_grp, P, grp*idx_w] snake layout:
    element (g0, p, g*idx_w + w) = flat[((g0*grp + g)*idx_w + w)*P + p]."""
    x = flat_idx32.reshape(n_grp, grp, idx_w, P)
    return np.ascontiguousarray(x.transpose(0, 3, 1, 2).reshape(n_grp, P, grp * idx_w))


def _host_reference(accept_index, out_cache_loc):
    # general fallback (handles negative indices); never hit for the given
    # input distribution
    size = accept_index.shape[0]
    mask = accept_index >= 0
    dst = np.cumsum(mask.astype(np.int64)) - 1
    src = np.maximum(accept_index, 0)
    vals = out_cache_loc[src]
    out = np.zeros((size,), dtype=out_cache_loc.dtype)
    out[dst[mask]] = vals[mask]
    return out


def kernel(accept_index: np.ndarray, out_cache_loc: np.ndarray) -> np.ndarray:
    accept_index = np.asarray(accept_index)
    out_cache_loc = np.asarray(out_cache_loc, dtype=np.float32)
    if accept_index.min() < 0:
        return _host_reference(accept_index, out_cache_loc)

    from concourse.bass_utils import run_bass_kernel_spmd

    idx32 = accept_index.astype(np.int32).reshape(N_CORES, SHARD)
    tab2d = out_cache_loc.reshape(N_CACHE, 1)

    nc = get_nc()
    in_maps = [{"idx": snake_idx(idx32[c]), "table": tab2d} for c in range(N_CORES)]
    res = run_bass_kernel_spmd(
        nc, in_maps, core_ids=list(range(N_CORES)), trace=TRACE
    )
    global LAST_RESULTS
    LAST_RESULTS = res
    out = np.concatenate(
        [res.results[c]["out"].reshape(-1) for c in range(N_CORES)], axis=0
    )
    return out

